# revision 4
# baseline (speedup 1.0000x reference)
"""Trainium2 Bass kernel for the nn_Circuit recurrence.

Math: a 7-state nonlinear EMA circuit scanned over T=2,000,000 steps:
    pv'  = 0.25*relu(Wffpv@stim + Wlat@pyr) + 0.75*pv
    pyr' = 0.1 *relu(Wffy @stim - Wiy@pv' + Wfby@hva) + 0.9*pyr
    hva' = 0.1 *relu(Wffh @pyr') + 0.9*hva
The recurrence forgets exponentially (empirical contraction ~0.94/step), so
the sequence is split into S = NCORES*P independent streams (one per SBUF
partition, F=1), each warmed up for W steps from a mean-state init using the
true preceding inputs.

The whole per-step chain runs on ONE engine (gpsimd/Pool) as a sequence of
single-column [128,1] ops: same-engine dependencies are enforced by program
order, so no cross-engine semaphore traffic.  Pool has no
scalar_tensor_tensor, so the EMA updates are made plain tensor_tensor adds by
keeping the state PRE-SCALED with a geometric growth that absorbs the decay:
within a renorm block of B steps, the state entering local step e is stored as
    P^ = pyr/0.9^e,  H^ = (hva_{k-1}/h_scale)/0.9^e,  X^ = (c_q*pv)/0.75^e
and every decay multiply folds into the relu's dual-scalar tensor_scalar or
into host-prescaled inputs.  Every B steps the state is renormalized back to
e=0.  The host rescales recorded outputs per step.

Everything (inputs, state, recorded outputs) is fp16 to halve DMA traffic;
input chunks and output segments are spread across the three DMA queues
(SP-HWDGE, Activation-HWDGE, Pool-SWDGE) so the transfers overlap.

Input per step (host precomputed): At'' = (Wffpv@stim)/(wlat*0.9^e) and
Bt'' = A_PYR*(Wffy@stim)/0.75^(e+1).
"""

import numpy as np

T_TOTAL = 2_000_000
NCORES = 8
P = 128

A_PV = np.float32(0.25)
A_PYR = np.float32(0.1)

MASK_FFY = np.array(
    [[1, 1, 0, 0, 0, 0], [0, 0, 1, 1, 0, 0], [0, 0, 0, 0, 1, 1]], np.float32
)
MASK_IY = np.array([[1, 0], [1, 1], [0, 1]], np.float32)
MASK_FFPV = np.array([[1, 1, 1, 0, 0, 0], [0, 0, 0, 1, 1, 1]], np.float32)
MASK_LAT = np.array([[1, 1, 0], [0, 1, 1]], np.float32)
MASK_FFH = np.ones((2, 3), np.float32)
MASK_FBY = np.ones((3, 2), np.float32)

# tunables
F = 1        # streams per partition (total S = NCORES*P)
WARM = 96    # warmup steps per stream (mean-init; contraction ~0.94/step)
NH = 1       # H state columns
NB = 32      # renorm block: state stored pre-scaled by 0.9^-e / 0.75^-e
NIN = 9      # input dma chunks (spread over the 3 queues)
NOUT = 9     # output dma segments (spread over the 3 queues)

NS = 5 + NH  # state slots per stream: [P0,P1,P2,H,X0,X1]
SW = NS      # state width per partition (F=1)

# state means for warm-start init (measured steady-state of the circuit)
MEAN_PYR = (0.613, 0.473, 0.602)
MEAN_PV = (0.815, 0.806)
MEAN_HVA = 1.687


def _patch_tile_drain():
    """This walrus build accepts at most ONE sync wait per instruction, but
    Tile's kernel-tail drain waits on every active proc at once.  Split it
    into a chain of single-wait drain instructions (SP executes in order, so
    the chain is semantically identical)."""
    import concourse.mybir as mybir
    from concourse import tile as _tile
    from concourse.vector_clock import ScopedClock

    if getattr(_tile.TileContext, "_drain_split_patched", False):
        return

    def _drain_and_barrier(self, tick_clock, wait_clock):
        drain_inst = self.nc.sync.drain()
        wait_clock.add_sem_waits(
            drain_inst.ins, ScopedClock({None: tick_clock.global_clock})
        )
        si = drain_inst.ins.sync_info
        if si is not None and si.on_wait and len(si.on_wait) > 1:
            waits = list(si.on_wait)
            upds = list(si.on_update) if si.on_update else []
            drain_inst.ins.sync_info = mybir.SyncInfo(
                on_wait=[waits[0]], on_update=[]
            )
            for w in waits[1:-1]:
                d = self.nc.sync.drain()
                d.ins.sync_info = mybir.SyncInfo(on_wait=[w], on_update=[])
            d = self.nc.sync.drain()
            d.ins.sync_info = mybir.SyncInfo(on_wait=[waits[-1]], on_update=upds)
        self.nc.all_engine_barrier()
        popped = self.nc._tile_sem_poison_stack.pop()
        assert popped is self._sem_poison
        self.nc.clear_and_free_semaphores(list(self.sems.allocated().values()))
        self.nc.all_engine_barrier()

    _tile.TileContext._drain_and_barrier = _drain_and_barrier
    _tile.TileContext._drain_split_patched = True


def _sc(e, c_lv, c_fb):
    """Per-local-step compile-time unit-conversion scalars."""
    cx = float(c_lv) * 0.9 ** e / 0.75 ** (e + 1)
    ch = 1.0 / 0.9
    chb = float(c_fb) * (0.9 / 0.75) ** (e + 1)
    cp = (0.75 / 0.9) ** (e + 1)
    return cx, ch, chb, cp


def _build_nc(F_unused, W, L, c_lv, c_fb, nh_unused):
    import concourse.bass as bass
    import concourse.mybir as mybir
    from contextlib import ExitStack
    from concourse.tile import TileContext

    _patch_tile_drain()

    AL = mybir.AluOpType
    f16 = mybir.dt.float16
    steps = W + L

    nc = bass.Bass(trn_type="TRN2", use_seq_codegen=True)
    X = nc.dram_tensor("x", [P, SW + steps * 5], f16, kind="ExternalInput")
    Y = nc.dram_tensor("y", [P, L * SW], f16, kind="ExternalOutput")

    with ExitStack() as ctx:
        tc = ctx.enter_context(TileContext(nc))
        spool = ctx.enter_context(tc.tile_pool(name="state", bufs=1))
        ST = spool.tile([P, L * SW], f16, name="ST")
        RS = spool.tile([P, 2 * SW], f16, name="RS")   # warmup ping-pong
        RNR = spool.tile([P, SW], f16, name="RNR")     # renormed state slot
        SC = spool.tile([P, 16], f16, name="SC")       # scratch cols
        ipool = ctx.enter_context(tc.tile_pool(name="inp", bufs=1))

        g = nc.gpsimd
        dma_engines = [nc.sync, nc.scalar, nc.gpsimd]
        qload = [0.0, 0.0, 0.0]  # queued bytes per partition, per queue

        def pick_queue(nbytes):
            qi = min(range(3), key=lambda i: qload[i])
            qload[qi] += nbytes
            return dma_engines[qi]

        # input DMAs: NIN chunks, all started up-front, spread across the 3
        # queues (least-loaded first).  Chunk 0 is small and carries the
        # SW-wide init-state block so compute can start quickly.
        sizes = [64]
        rem = steps - 64
        for c in range(NIN - 1):
            n = (rem + NIN - 2 - c) // (NIN - 1 - c)
            sizes.append(n)
            rem -= n
        sizes = [n for n in sizes if n > 0]
        bounds = [0]
        for n in sizes:
            bounds.append(bounds[-1] + n)
        in_tiles = []
        for c, n in enumerate(sizes):
            pad = SW if c == 0 else 0
            t = ipool.tile([P, pad + n * 5], f16, name=f"inchunk{c}")
            lo = 0 if c == 0 else SW + bounds[c] * 5
            hi = SW + bounds[c + 1] * 5
            eng = pick_queue((pad + n * 5) * 2)
            eng.dma_start(out=t[:, :], in_=X[:, lo:hi])
            in_tiles.append(t)

        def chunk_of(k):
            for c in range(len(sizes)):
                if k < bounds[c + 1]:
                    return in_tiles[c], (k - bounds[c]) * 5 + (SW if c == 0 else 0)
            raise AssertionError

        def slot(k):
            # state location after step k (k = -1 is the DMA'd init block)
            if k < 0:
                return in_tiles[0][:, 0:SW]
            if k < W:
                o = (k % 2) * SW
                return RS[:, o : o + SW]
            j = k - W
            return ST[:, j * SW : (j + 1) * SW]

        # scratch column aliases (all [P,1])
        S2a = SC[:, 0:1]
        S2b = SC[:, 1:2]
        G0 = SC[:, 2:3]
        G1 = SC[:, 3:4]
        S3 = SC[:, 4:5]
        RX0 = SC[:, 5:6]
        RX1 = SC[:, 6:7]
        RH = SC[:, 7:8]
        HB = SC[:, 8:9]
        Xs = SC[:, 9:10]
        U0 = SC[:, 10:11]
        U1 = SC[:, 11:12]
        U2 = SC[:, 12:13]

        # output segment boundaries (in output-step space), last segment small
        fr = [i / (NOUT - 0.75) for i in range(NOUT)] + [1.0]
        oseg = sorted({round(f * L) for f in fr})
        oseg_i = 0

        for k in range(steps):
            e = k % NB
            cx, ch, chb, cp = _sc(e, c_lv, c_fb)
            prev = RNR[:, :] if (k > 0 and e == 0) else slot(k - 1)
            cur = slot(k)
            it, off = chunk_of(k)
            At0 = it[:, off : off + 1]
            At1 = it[:, off + 1 : off + 2]
            Bt0 = it[:, off + 2 : off + 3]
            Bt1 = it[:, off + 3 : off + 4]
            Bt2 = it[:, off + 4 : off + 5]
            pP0, pP1, pP2 = prev[:, 0:1], prev[:, 1:2], prev[:, 2:3]
            pH, pX0, pX1 = prev[:, 3:4], prev[:, 4:5], prev[:, 5:6]
            cP0, cP1, cP2 = cur[:, 0:1], cur[:, 1:2], cur[:, 2:3]
            cH, cX0, cX1 = cur[:, 3:4], cur[:, 4:5], cur[:, 5:6]

            # prev-pyr sums: S2 = [P0+P1, P1+P2]; S3 = P0+P1+P2
            g.tensor_tensor(S2a, pP0, pP1, AL.add)
            g.tensor_tensor(S2b, pP1, pP2, AL.add)
            g.tensor_tensor(S3, S2a, pP2, AL.add)
            # pv drive + relu with unit conversion
            g.tensor_tensor(G0, S2a, At0, AL.add)
            g.tensor_tensor(G1, S2b, At1, AL.add)
            g.tensor_scalar(RX0, G0, 0.0, cx, AL.max, AL.mult)
            g.tensor_scalar(RX1, G1, 0.0, cx, AL.max, AL.mult)
            # EMAs as plain adds (pre-scaled state)
            g.tensor_tensor(cX0, pX0, RX0, AL.add)
            g.tensor_tensor(cX1, pX1, RX1, AL.add)
            # hva drive: pyr >= 0 always so relu(S3) = S3
            g.tensor_scalar(RH, S3, ch, None, AL.mult)
            g.tensor_tensor(cH, pH, RH, AL.add)
            # feedback column (shared by all 3 pyr rows)
            g.tensor_scalar(HB, cH, chb, None, AL.mult)
            # pyr drive: U_c = Bt_c - Wiy@pv' + HB
            g.tensor_tensor(Xs, cX0, cX1, AL.add)
            g.tensor_tensor(U0, Bt0, cX0, AL.subtract)
            g.tensor_tensor(U1, Bt1, Xs, AL.subtract)
            g.tensor_tensor(U2, Bt2, cX1, AL.subtract)
            g.tensor_tensor(U0, U0, HB, AL.add)
            g.tensor_tensor(U1, U1, HB, AL.add)
            g.tensor_tensor(U2, U2, HB, AL.add)
            # relu with unit conversion, P' EMA
            g.tensor_scalar(U0, U0, 0.0, cp, AL.max, AL.mult)
            g.tensor_scalar(U1, U1, 0.0, cp, AL.max, AL.mult)
            g.tensor_scalar(U2, U2, 0.0, cp, AL.max, AL.mult)
            g.tensor_tensor(cP0, pP0, U0, AL.add)
            g.tensor_tensor(cP1, pP1, U1, AL.add)
            g.tensor_tensor(cP2, pP2, U2, AL.add)

            # renorm every NB steps: back to local exponent 0
            if (k + 1) % NB == 0 and k + 1 < steps:
                for c in range(4):
                    g.tensor_scalar(
                        RNR[:, c : c + 1], cur[:, c : c + 1], 0.9**NB, None, AL.mult
                    )
                for c in range(4, 6):
                    g.tensor_scalar(
                        RNR[:, c : c + 1], cur[:, c : c + 1], 0.75**NB, None, AL.mult
                    )

            # stream finished output segments out while the loop continues
            if k >= W and oseg_i < len(oseg) - 1 and (k - W + 1) == oseg[oseg_i + 1]:
                lo, hi = oseg[oseg_i], oseg[oseg_i + 1]
                eng = pick_queue((hi - lo) * SW * 2)
                eng.dma_start(
                    out=Y[:, lo * SW : hi * SW], in_=ST[:, lo * SW : hi * SW]
                )
                oseg_i += 1

    return nc


def _prep_inputs(I, Wffpv, Wffy, wlat, W, L):
    """Per-core DRAM input arrays (P, SW + steps*5), fp16, laid out
    [init(SW)] [step][At0,At1,Bt0,Bt1,Bt2], with the per-step renorm
    pre-scaling folded in."""
    S = NCORES * P
    steps = W + L
    Aff = I @ Wffpv.T.astype(np.float32)          # (T,2)
    Bff = (I @ Wffy.T.astype(np.float32)) * A_PYR  # (T,3)
    FF = np.concatenate([Aff, Bff], axis=1).astype(np.float32)  # (T,5)

    FFp = np.zeros((W + S * L, 5), np.float32)
    FFp[W : W + T_TOTAL] = FF
    sv = np.lib.stride_tricks.as_strided(
        FFp,
        shape=(S, steps, 5),
        strides=(L * FFp.strides[0], FFp.strides[0], FFp.strides[1]),
    )
    arr = sv.copy()  # (S, steps, 5)
    # fold per-step unit scales: At'' = Aff/(wlat*0.9^e); Bt'' = Bt/0.75^(e+1)
    e = np.arange(steps) % NB
    arr[:, :, 0:2] /= (np.float32(wlat) * 0.9**e)[None, :, None].astype(np.float32)
    arr[:, :, 2:5] /= (0.75 ** (e + 1))[None, :, None].astype(np.float32)

    # stream s = core*P + p  ->  core-local (P, steps*5)
    arr = arr.reshape(NCORES, P, steps * 5).astype(np.float16)

    # init block: mean state (true units, e=0), stream 0 starts from zeros;
    # the unit-dependent H/Xv lanes are overwritten by the caller.
    init = np.empty((NCORES, P, NS), np.float32)
    init[..., 0] = MEAN_PYR[0]
    init[..., 1] = MEAN_PYR[1]
    init[..., 2] = MEAN_PYR[2]
    init[..., 3] = MEAN_HVA  # overwritten by caller (unit-dependent)
    init[..., 4] = MEAN_PV[0]  # overwritten by caller
    init[..., 5] = MEAN_PV[1]
    return arr, init


def _assemble_output(outs, c_q, h_scale, W, L):
    """outs: per-core (P, L*SW) pre-scaled recorded states -> (7, T)."""
    Y = np.stack(outs).astype(np.float32)  # (NCORES, P, L*SW)
    Y = Y.reshape(NCORES * P, L, NS)
    # undo the renorm pre-scaling: output of step j recorded at exponent
    # ((W + j) % NB) + 1
    j = np.arange(L)
    x = ((W + j) % NB) + 1
    s9 = (0.9**x).astype(np.float32)
    s75 = (0.75**x).astype(np.float32)
    res7 = np.empty((NCORES * P, L, 7), np.float32)
    res7[:, :, 0:3] = Y[:, :, 0:3] * s9[None, :, None]
    res7[:, :, 3:5] = Y[:, :, 4:6] * (s75 / np.float32(c_q))[None, :, None]
    res7[:, :, 5] = Y[:, :, 3] * (s9 * np.float32(h_scale))[None, :]
    res7[:, :, 6] = res7[:, :, 5]
    return np.ascontiguousarray(res7.reshape(-1, 7)[:T_TOTAL].T)


def _mask_weights(W_FFpv, W_LatPV, W_FFy, W_Iy, W_FFh, W_FBy):
    return (
        np.maximum(np.asarray(W_FFpv, np.float32), 0) * MASK_FFPV,
        np.maximum(np.asarray(W_LatPV, np.float32), 0) * MASK_LAT,
        np.maximum(np.asarray(W_FFy, np.float32), 0) * MASK_FFY,
        np.maximum(np.asarray(W_Iy, np.float32), 0) * MASK_IY,
        np.maximum(np.asarray(W_FFh, np.float32), 0) * MASK_FFH,
        np.maximum(np.asarray(W_FBy, np.float32), 0) * MASK_FBY,
    )


def _uniform(vals):
    vals = np.asarray(vals)
    return vals.size > 0 and np.all(vals == vals.flat[0])


def _numpy_fallback(I, Wffpv, Wlat, Wffy, Wiy, Wffh, Wfby, W=1024):
    """General (non-uniform-weight) streamed scan, numpy only."""
    S = 4096
    L = (T_TOTAL + S - 1) // S
    steps = W + L
    Aff = (I @ Wffpv.T).astype(np.float32)
    Bff = (I @ Wffy.T).astype(np.float32)
    FF = np.concatenate([Aff, Bff], axis=1)
    FFp = np.zeros((W + S * L, 5), np.float32)
    FFp[W : W + T_TOTAL] = FF
    sv = np.lib.stride_tricks.as_strided(
        FFp,
        shape=(S, steps, 5),
        strides=(L * FFp.strides[0], FFp.strides[0], FFp.strides[1]),
    )
    Xs = np.ascontiguousarray(sv)
    pyr = np.zeros((S, 3), np.float32)
    pv = np.zeros((S, 2), np.float32)
    hva = np.zeros((S, 2), np.float32)
    out = np.zeros((S, L, 7), np.float32)
    WlatT = Wlat.T.astype(np.float32)
    WiyT = Wiy.T.astype(np.float32)
    WffhT = Wffh.T.astype(np.float32)
    WfbyT = Wfby.T.astype(np.float32)
    for k in range(steps):
        a = Xs[:, k, 0:2]
        b = Xs[:, k, 2:5]
        pv = A_PV * np.maximum(a + pyr @ WlatT, 0) + (1 - A_PV) * pv
        pyr_n = (
            A_PYR * np.maximum(b - pv @ WiyT + hva @ WfbyT, 0) + (1 - A_PYR) * pyr
        )
        hva_n = A_PYR * np.maximum(pyr_n @ WffhT, 0) + (1 - A_PYR) * hva
        if k >= W:
            out[:, k - W, 0:3] = pyr_n
            out[:, k - W, 3:5] = pv
            out[:, k - W, 5:7] = hva
        pyr, hva = pyr_n, hva_n
    return np.ascontiguousarray(out.reshape(S * L, 7)[:T_TOTAL].T)


def kernel(I, W_FFpv, W_LatPV, W_FFy, W_Iy, W_FFh, W_FBy):
    I = np.asarray(I, np.float32)
    Wffpv, Wlat, Wffy, Wiy, Wffh, Wfby = _mask_weights(
        W_FFpv, W_LatPV, W_FFy, W_Iy, W_FFh, W_FBy
    )

    wlat = Wlat[0, 0]
    wiy = Wiy[0, 0]
    wffh = Wffh[0, 0]
    wfby = Wfby[0, 0]
    fast = (
        _uniform(Wlat[MASK_LAT > 0])
        and _uniform(Wiy[MASK_IY > 0])
        and _uniform(Wffh)
        and _uniform(Wfby)
        and wffh > 0
        and wiy > 0
        and wlat > 0
    )
    if not fast:
        return _numpy_fallback(I, Wffpv, Wlat, Wffy, Wiy, Wffh, Wfby)

    c_q = np.float32(A_PYR * wiy)       # Xv = c_q * pv
    h_scale = np.float32(A_PYR * wffh)  # hva = h_scale * H (delayed)
    c_lv = np.float32(c_q * A_PV * wlat)
    c_fb = np.float32(A_PYR * wfby * 2.0 * h_scale)

    S = NCORES * P
    L = (T_TOTAL + S - 1) // S

    try:
        from concourse.bass_utils import run_bass_kernel_spmd

        nc = _build_nc(F, WARM, L, float(c_lv), float(c_fb), NH)
        arr, init = _prep_inputs(I, Wffpv, Wffy, wlat, WARM, L)
        # init block in true pre-scaled units (e=0): [P0,P1,P2,H,X0,X1]
        init[..., 3] = np.float32(MEAN_HVA / h_scale)
        init[..., 4] = np.float32(c_q * MEAN_PV[0])
        init[..., 5] = np.float32(c_q * MEAN_PV[1])
        init[0, 0, :] = 0.0  # stream 0 = true zero start
        init = init.astype(np.float16)
        xs = [
            np.concatenate([init[c], arr[c]], axis=1).astype(np.float16)
            for c in range(NCORES)
        ]
        res = run_bass_kernel_spmd(
            nc, [{"x": x} for x in xs], core_ids=list(range(NCORES))
        )
        outs = [res.results[c]["y"] for c in range(NCORES)]
        return _assemble_output(outs, c_q, h_scale, WARM, L)
    except Exception:
        return _numpy_fallback(I, Wffpv, Wlat, Wffy, Wiy, Wffh, Wfby)
